# revision 9
# baseline (speedup 1.0000x reference)
"""Causal self-attention (B=4, T=2048, C=1024, 16 heads) on 8 trn2 NeuronCores.

Sharding: core c -> (batch b = c//2, head-half hh = c%2). Each core computes
one batch x 8 heads: QKV column-parallel + out-proj row-parallel (Megatron);
the host sums the two partial outputs per batch. No collectives.

Schedule (single continuous PE stream to keep the HAM clock at 2.4 GHz):
  - DMA ordered so v/qk projections start as soon as the first x t-block lands
  - exp table preloaded at t~0 via a dummy activation
  - attention per head: scores (transposed, causal-trimmed) -> exp on ACT ->
    y accumulated in 512-col "quarters"; next head's scores interleave into
    the last quarter's tk passes so the single pT buffer recycles per region
  - qk projections for later head-pairs are emitted as PE filler inside the
    attention phase (covers the ACT exp deficit)
  - out-projection pairs heads (2m, 2m+1) into a K=128 contraction
  - all matmuls bf16 (1 cycle/row); rowsum-reciprocal broadcast via a K=1
    bf16 matmul

Self-contained: hardcodes shapes; builds/compiles the Bass program once per
process and runs it via run_bass_kernel_spmd on cores 0-7.
"""

import numpy as np
import ml_dtypes

B, T, C = 4, 2048, 1024
N_HEAD = 16
D = 64          # head dim
NHC = 8         # heads per core
CC = 512        # channels per core (NHC * D)
KO = 8          # contraction chunks of 128 over C
TM = 16         # t chunks of 128

_NC = None          # cached compiled Bass program
LAST_RESULTS = None  # BassKernelResults of the last run (for test harness)


def _off(i):
    # start offset of score tile i inside the packed pT tensor
    return 2048 * i - 64 * i * (i - 1)


PT_LEN = _off(16)  # 17408


def build_nc():
    import concourse.bacc as bacc
    import concourse.mybir as mybir
    import concourse.tile as tile
    from concourse.masks import make_upper_triangular
    from contextlib import ExitStack

    bf16 = mybir.dt.bfloat16
    f32 = mybir.dt.float32
    EXP = mybir.ActivationFunctionType.Exp

    nc = bacc.Bacc("TRN2", target_bir_lowering=False, debug=False)

    xT = nc.dram_tensor("xT", [C, T], bf16, kind="ExternalInput")
    wq = nc.dram_tensor("wqT", [C, CC], bf16, kind="ExternalInput")
    wk = nc.dram_tensor("wkT", [C, CC], bf16, kind="ExternalInput")
    wv = nc.dram_tensor("wvT", [C, CC], bf16, kind="ExternalInput")
    wp = nc.dram_tensor("wpT", [CC, C], bf16, kind="ExternalInput")
    out = nc.dram_tensor("out", [T, C], f32, kind="ExternalOutput")

    with tile.TileContext(nc) as tc, ExitStack() as ctx:
        const = ctx.enter_context(tc.tile_pool(name="const", bufs=1))
        warm_i = const.tile([128, 8], f32)
        warm_o = const.tile([128, 8], f32)
        mask_sb = const.tile([128, 128], bf16)
        ones_bf = const.tile([128, 128], bf16)

        # pull the exp table load to t~0 (it costs ~2.7us on first use)
        nc.gpsimd.memset(warm_i[0:1, :], 0.0)
        nc.scalar.activation(warm_o[0:1, :], warm_i[0:1, :], EXP)
        # keep-mask for the diagonal 128x128 block of p.T tiles: 1 where tq>=tk
        make_upper_triangular(nc, mask_sb[:], val=1.0, diag=True)
        nc.gpsimd.memset(ones_bf[0:1, :], 1.0)

        wts = ctx.enter_context(tc.tile_pool(name="wts", bufs=1))
        xT_sb = wts.tile([128, KO, T], bf16)
        wv_sb = wts.tile([128, KO, CC], bf16)
        wp_sb = wts.tile([128, 4, C], bf16)

        data = ctx.enter_context(tc.tile_pool(name="data", bufs=1))
        qT_sb = data.tile([128, 4, T], bf16)
        kT_sb = data.tile([128, 4, T], bf16)
        # v2[p, ti, l, 0:128] = [ones(64) | v_l(64)] for every head l: the y
        # matmul then yields the softmax denominator at psum rows 0:64 and y
        # at rows 64:128 in one pass
        v2_sb = data.tile([128, TM, NHC, 128], bf16)
        # normalized y, head-paired: head h lives at partitions 64*(h%2) in
        # slot m=h//2, giving a K=128 contraction for the out-projection
        yTn_sb = data.tile([128, 4, T], bf16)
        pT = data.tile([128, PT_LEN], bf16)
        # out-proj partial sums over head-pairs 0..2 for t-tiles 0..7,
        # accumulated as PE filler during heads 6-7 (pairs 0-2 are final then)
        os_part = data.tile([128, 8, 2, 512], bf16)

        nc.gpsimd.memset(v2_sb[:, :, :, 0:64], 1.0)

        # DMA: priority order so compute can start early (first PE work is
        # qk_proj(0) n=0 which needs only wq/wk m=0 + x t-block 0)
        wqk = ctx.enter_context(tc.tile_pool(name="wqk", bufs=2))
        wm_tiles = {}

        def dma_wqk(m):
            for w_dram, tg in ((wq, "wq"), (wk, "wk")):
                wm = wqk.tile([128, KO, 128], bf16, name=f"{tg}{m}", tag=tg)
                nc.sync.dma_start(
                    wm[:],
                    w_dram.rearrange("(ko p) d -> p ko d", p=128)[
                        :, :, 128 * m : 128 * m + 128
                    ],
                )
                wm_tiles[(m, tg)] = wm

        dma_wqk(0)
        xTr = xT.rearrange("(ko p) t -> p ko t", p=128)
        for k in range(KO):
            nc.sync.dma_start(xT_sb[:, k, 0:512], xTr[:, k, 0:512])
        nc.sync.dma_start(wv_sb[:], wv.rearrange("(ko p) d -> p ko d", p=128))
        for tb in range(1, 4):
            for k in range(KO):
                nc.sync.dma_start(
                    xT_sb[:, k, 512 * tb : 512 * tb + 512],
                    xTr[:, k, 512 * tb : 512 * tb + 512],
                )
        nc.sync.dma_start(wp_sb[:], wp.rearrange("(m p) e -> p m e", p=128))

        pj = ctx.enter_context(tc.tile_pool(name="pj", bufs=2, space="PSUM"))
        sT_ps = ctx.enter_context(tc.tile_pool(name="sT_ps", bufs=2, space="PSUM"))
        yq_ps = ctx.enter_context(tc.tile_pool(name="yq_ps", bufs=2, space="PSUM"))

        norm = ctx.enter_context(tc.tile_pool(name="norm", bufs=2))
        ost = ctx.enter_context(tc.tile_pool(name="ost", bufs=2))

        def v_proj(mt):
            ps = pj.tile([128, 512], f32, name=f"psv{mt}", tag="pj")
            for k in range(KO):
                nc.tensor.matmul(
                    ps[:],
                    lhsT=xT_sb[:, k, 128 * mt : 128 * mt + 128],
                    rhs=wv_sb[:, k],
                    start=(k == 0),
                    stop=(k == KO - 1),
                )
            # scatter per-head 64-col blocks into v2 (even l -> cols 0:64,
            # odd l -> cols 64:128)
            psv = ps.rearrange("p (l c) -> p l c", c=64)
            nc.vector.tensor_copy(v2_sb[:, mt, :, 64:128], psv[:])

        def qk_group(m, tg, n):
            wm = wm_tiles[(m, tg)]
            o_sb = qT_sb if tg == "wq" else kT_sb
            ps = pj.tile([128, 512], f32, name=f"psqk{tg}{m}{n}", tag="pj")
            for k in range(KO):
                nc.tensor.matmul(
                    ps[:],
                    lhsT=wm[:, k, :],
                    rhs=xT_sb[:, k, 512 * n : 512 * n + 512],
                    start=(k == 0),
                    stop=(k == KO - 1),
                )
            nc.vector.tensor_copy(o_sb[:, m, 512 * n : 512 * n + 512], ps[:])

        def score_tile(h, i):
            hm, hp = h // 2, h % 2
            pb = 64 * hp
            off = _off(i)
            W = T - 128 * i
            for c0 in range(0, W, 1024):
                Wc = min(1024, W - c0)
                st = sT_ps.tile([128, 1024], f32, name=f"st{h}_{i}_{c0}", tag="sT")
                for s0 in range(0, Wc, 512):
                    Ws = min(512, Wc - s0)
                    nc.tensor.matmul(
                        st[:, s0 : s0 + Ws],
                        lhsT=kT_sb[pb : pb + 64, hm, 128 * i : 128 * i + 128],
                        rhs=qT_sb[pb : pb + 64, hm, 128 * i + c0 + s0 :][:, :Ws],
                        start=True,
                        stop=True,
                    )
                nc.scalar.activation(
                    pT[:, off + c0 : off + c0 + Wc], st[:, :Wc], EXP, scale=0.125
                )
                if c0 == 0:
                    # causal mask inside the diagonal 128x128 block
                    nc.vector.tensor_mul(
                        pT[:, off : off + 128], pT[:, off : off + 128], mask_sb[:]
                    )

        # filler queue: qk projections for m=1..3, one (w, n) group per slot
        filler = [(m, tg, n) for m in (1, 2, 3) for n in range(4) for tg in ("wq", "wk")]
        fi = 0

        def out_stage(mt, n):
            # partial out-proj: pairs m=0..2 -> bf16 staging (pair 3 + add later)
            ps = pj.tile([128, 512], f32, name=f"pst{mt}{n}", tag="pj")
            for m in range(3):
                nc.tensor.matmul(
                    ps[:],
                    lhsT=yTn_sb[:, m, 128 * mt : 128 * mt + 128],
                    rhs=wp_sb[:, m, 512 * n : 512 * n + 512],
                    start=(m == 0),
                    stop=(m == 2),
                )
            nc.vector.tensor_copy(os_part[:, mt, n, :], ps[:])

        stage_q = [(mt, n) for mt in range(8) for n in range(2)]
        si = 0

        # ---- phase 1: v proj + qk proj m=0, per t-block as DMA lands ----
        for tb in range(4):
            qk_group(0, "wq", tb)
            qk_group(0, "wk", tb)
            if tb < 2:
                for mt in range(4 * tb, 4 * tb + 4):
                    v_proj(mt)

        # ---- scores for head 0, v proj mt 8..15 interleaved as filler ----
        defer_v = list(range(8, 16))
        for i in range(16):
            score_tile(0, i)
            if i % 2 == 0 and defer_v:
                v_proj(defer_v.pop(0))

        outr = out.rearrange("(mt p) e -> p mt e", p=128)

        # ---- attention heads ----
        for h in range(NHC):
            m = h // 2
            if h % 2 == 0 and m + 1 < 4:
                dma_wqk(m + 1)
            for q in range(4):
                # PE filler (also: scores h+1 below read qT/kT of pair m+1,
                # so every m+1 group must be emitted before h's q3 i-loop)
                if fi < len(filler) and h < 6:
                    qk_group(*filler[fi])
                    fi += 1
                elif h >= 6:
                    for _ in range(2):
                        if si < len(stage_q):
                            out_stage(*stage_q[si])
                            si += 1
                ilim = min(4 * q + 4, 16)
                yq = yq_ps.tile([128, 512], f32, name=f"yq{h}{q}", tag="yq")
                for i in range(ilim):
                    c0 = max(0, 128 * i - 512 * q)  # col offset inside quarter
                    w = 512 - c0
                    po = _off(i) + 512 * q + c0 - 128 * i
                    nc.tensor.matmul(
                        yq[:, c0:512],
                        lhsT=v2_sb[:, i, h, :],
                        rhs=pT[:, po : po + w],
                        start=(i == 0),
                        stop=(i == ilim - 1),
                    )
                    if q == 3 and h < NHC - 1:
                        score_tile(h + 1, i)
                # normalize: y rows 64:128, rowsum replicated at rows 0:64
                ri = norm.tile([128, 512], f32, name=f"ri{h}{q}", tag="ri")
                sc = norm.tile([128, 512], f32, name=f"sc{h}{q}", tag="sc")
                rib = norm.tile([128, 512], bf16, name=f"rib{h}{q}", tag="rib")
                rbs = norm.tile([128, 512], f32, name=f"rbs{h}{q}", tag="rbs")
                nc.vector.reciprocal_approx_accurate(
                    out=ri[0:64, :], in_=yq[0:64, :], scratch=sc[0:64, :]
                )
                nc.vector.tensor_copy(rib[0:1, :], ri[0:1, :])
                # broadcast the reciprocal row to all partitions via a K=1
                # bf16 matmul (ones column x recip row), staged through PSUM
                rb_ps = pj.tile([128, 512], f32, name=f"rbp{h}{q}", tag="pj")
                nc.tensor.matmul(
                    rb_ps[:],
                    lhsT=ones_bf[0:1, :],
                    rhs=rib[0:1, :],
                    start=True,
                    stop=True,
                )
                nc.vector.tensor_copy(rbs[:], rb_ps[:])
                cols = slice(512 * q, 512 * q + 512)
                if h % 2 == 1:
                    nc.vector.tensor_mul(
                        yTn_sb[64:128, m, cols], yq[64:128, :], rbs[64:128, :]
                    )
                else:
                    # even heads land at partitions 0:64 of the paired layout;
                    # engines can't shift partitions, so stage + SBUF DMA
                    yt_h = norm.tile([128, 512], bf16, name=f"ytmp{h}{q}", tag="ytmp")
                    nc.vector.tensor_mul(
                        yt_h[64:128, :], yq[64:128, :], rbs[64:128, :]
                    )
                    nc.sync.dma_start(yTn_sb[0:64, m, cols], yt_h[64:128, :])

        # ---- out projection: out[t, e] = y @ Wp_sub^T, head-paired K=128.
        # t-tiles 0..7: pair-3 matmul + staged partials; 8..15: all 4 pairs.
        # psum->sbuf copies alternate scalar/vector so neither engine binds.
        for mt in range(TM):
            for n in range(2):
                ps = pj.tile([128, 512], f32, name=f"pso{mt}{n}", tag="pj")
                m0 = 3 if mt < 8 else 0
                for m in range(m0, 4):
                    nc.tensor.matmul(
                        ps[:],
                        lhsT=yTn_sb[:, m, 128 * mt : 128 * mt + 128],
                        rhs=wp_sb[:, m, 512 * n : 512 * n + 512],
                        start=(m == m0),
                        stop=(m == 3),
                    )
                o_sb = ost.tile([128, 512], f32, name=f"ost{mt}{n}", tag="ost")
                if mt < 8:
                    nc.vector.tensor_add(o_sb[:], ps[:], os_part[:, mt, n, :])
                elif (2 * mt + n) % 2:
                    nc.vector.tensor_copy(o_sb[:], ps[:])
                else:
                    nc.scalar.copy(o_sb[:], ps[:])
                nc.sync.dma_start(outr[:, mt, 512 * n : 512 * n + 512], o_sb[:])

    nc.compile()
    return nc


def _get_nc():
    global _NC
    if _NC is None:
        _NC = build_nc()
    return _NC


def kernel(x, Wk, Wq, Wv, Wp, _trace=False):
    from concourse.bass_utils import run_bass_kernel_spmd

    global LAST_RESULTS
    bf16 = ml_dtypes.bfloat16
    x = np.asarray(x, dtype=np.float32)
    Wk = np.asarray(Wk, dtype=np.float32)
    Wq = np.asarray(Wq, dtype=np.float32)
    Wv = np.asarray(Wv, dtype=np.float32)
    Wp = np.asarray(Wp, dtype=np.float32)

    in_maps = []
    for c in range(8):
        b, hh = c // 2, c % 2
        cols = slice(CC * hh, CC * hh + CC)
        in_maps.append(
            {
                "xT": np.ascontiguousarray(x[b].T).astype(bf16),
                "wqT": np.ascontiguousarray(Wq[cols, :].T).astype(bf16),
                "wkT": np.ascontiguousarray(Wk[cols, :].T).astype(bf16),
                "wvT": np.ascontiguousarray(Wv[cols, :].T).astype(bf16),
                "wpT": np.ascontiguousarray(Wp[:, cols].T).astype(bf16),
            }
        )

    nc = _get_nc()
    res = run_bass_kernel_spmd(nc, in_maps, core_ids=list(range(8)), trace=_trace)
    LAST_RESULTS = res

    out = np.empty((B, T, C), dtype=np.float32)
    for b in range(B):
        out[b] = res.results[2 * b]["out"] + res.results[2 * b + 1]["out"]
    return out


if __name__ == "__main__":
    rng = np.random.default_rng(0)
    s = 1.0 / np.sqrt(C)
    inputs = {
        "x": rng.standard_normal((B, T, C), dtype=np.float32),
        "Wk": rng.standard_normal((C, C), dtype=np.float32) * s,
        "Wq": rng.standard_normal((C, C), dtype=np.float32) * s,
        "Wv": rng.standard_normal((C, C), dtype=np.float32) * s,
        "Wp": rng.standard_normal((C, C), dtype=np.float32) * s,
    }
    got = kernel(**inputs)

    def ref(x, Wk, Wq, Wv, Wp):
        def heads(w):
            return (
                np.einsum("btc,ec->bte", x, w)
                .reshape(B, T, N_HEAD, D)
                .transpose(0, 2, 1, 3)
            )

        k, q, v = heads(Wk), heads(Wq), heads(Wv)
        att = np.einsum("bhqd,bhkd->bhqk", q, k) / np.sqrt(D)
        causal = np.tril(np.ones((T, T), dtype=bool))
        att = np.where(causal[None, None], att, -np.inf)
        att = att - att.max(axis=-1, keepdims=True)
        e = np.exp(att)
        p = e / e.sum(axis=-1, keepdims=True)
        y = np.einsum("bhqk,bhkd->bhqd", p, v)
        y = y.transpose(0, 2, 1, 3).reshape(B, T, C)
        return np.einsum("btc,ec->bte", y, Wp)

    want = ref(**{k: v.astype(np.float64) for k, v in inputs.items()}).astype(
        np.float32
    )
    rel = np.linalg.norm(got - want) / np.linalg.norm(want)
    print("rel l2 err:", rel)
    print("max abs err:", np.abs(got - want).max(), "ref absmax:", np.abs(want).max())


# revision 10
# speedup vs baseline: 1.0248x; 1.0248x over previous
"""Causal self-attention (B=4, T=2048, C=1024, 16 heads) on 8 trn2 NeuronCores.

Sharding: core c -> (batch b = c//2, head-half hh = c%2). Each core computes
one batch x 8 heads: QKV column-parallel + out-proj row-parallel (Megatron);
the host sums the two partial outputs per batch. No collectives.

Schedule (single continuous PE stream to keep the HAM clock at 2.4 GHz):
  - DMA ordered so v/qk projections start as soon as the first x t-block lands
  - exp table preloaded at t~0 via a dummy activation
  - attention per head: scores (transposed, causal-trimmed) -> exp on ACT ->
    y accumulated in 512-col "quarters"; next head's scores interleave into
    the last quarter's tk passes so the single pT buffer recycles per region
  - qk projections for later head-pairs are emitted as PE filler inside the
    attention phase (covers the ACT exp deficit)
  - out-projection pairs heads (2m, 2m+1) into a K=128 contraction
  - all matmuls bf16 (1 cycle/row); rowsum-reciprocal broadcast via a K=1
    bf16 matmul

Self-contained: hardcodes shapes; builds/compiles the Bass program once per
process and runs it via run_bass_kernel_spmd on cores 0-7.
"""

import numpy as np
import ml_dtypes

B, T, C = 4, 2048, 1024
N_HEAD = 16
D = 64          # head dim
NHC = 8         # heads per core
CC = 512        # channels per core (NHC * D)
KO = 8          # contraction chunks of 128 over C
TM = 16         # t chunks of 128

_NC = None          # cached compiled Bass program
LAST_RESULTS = None  # BassKernelResults of the last run (for test harness)


def _off(i):
    # start offset of score tile i inside the packed pT tensor
    return 2048 * i - 64 * i * (i - 1)


PT_LEN = _off(16)  # 17408


def build_nc():
    import concourse.bacc as bacc
    import concourse.mybir as mybir
    import concourse.tile as tile
    from concourse.masks import make_upper_triangular
    from contextlib import ExitStack

    bf16 = mybir.dt.bfloat16
    f32 = mybir.dt.float32
    EXP = mybir.ActivationFunctionType.Exp

    nc = bacc.Bacc("TRN2", target_bir_lowering=False, debug=False)

    xT = nc.dram_tensor("xT", [C, T], bf16, kind="ExternalInput")
    wq = nc.dram_tensor("wqT", [C, CC], bf16, kind="ExternalInput")
    wk = nc.dram_tensor("wkT", [C, CC], bf16, kind="ExternalInput")
    wv = nc.dram_tensor("wvT", [C, CC], bf16, kind="ExternalInput")
    wp = nc.dram_tensor("wpT", [CC, C], bf16, kind="ExternalInput")
    out = nc.dram_tensor("out", [T, C], f32, kind="ExternalOutput")

    with tile.TileContext(nc) as tc, ExitStack() as ctx:
        const = ctx.enter_context(tc.tile_pool(name="const", bufs=1))
        warm_i = const.tile([128, 8], f32)
        warm_o = const.tile([128, 8], f32)
        mask_sb = const.tile([128, 128], bf16)
        ones_bf = const.tile([128, 128], bf16)

        # pull the exp table load to t~0 (it costs ~2.7us on first use)
        nc.gpsimd.memset(warm_i[0:1, :], 0.0)
        nc.scalar.activation(warm_o[0:1, :], warm_i[0:1, :], EXP)
        # keep-mask for the diagonal 128x128 block of p.T tiles: 1 where tq>=tk
        make_upper_triangular(nc, mask_sb[:], val=1.0, diag=True)
        nc.gpsimd.memset(ones_bf[0:1, :], 1.0)

        wts = ctx.enter_context(tc.tile_pool(name="wts", bufs=1))
        xT_sb = wts.tile([128, KO, T], bf16)
        wv_sb = wts.tile([128, KO, CC], bf16)
        wp_sb = wts.tile([128, 4, C], bf16)

        data = ctx.enter_context(tc.tile_pool(name="data", bufs=1))
        qT_sb = data.tile([128, 4, T], bf16)
        kT_sb = data.tile([128, 4, T], bf16)
        # v2[p, ti, l, 0:128] = [ones(64) | v_l(64)] for every head l: the y
        # matmul then yields the softmax denominator at psum rows 0:64 and y
        # at rows 64:128 in one pass
        v2_sb = data.tile([128, TM, NHC, 128], bf16)
        # normalized y, head-paired: head h lives at partitions 64*(h%2) in
        # slot m=h//2, giving a K=128 contraction for the out-projection
        yTn_sb = data.tile([128, 4, T], bf16)
        pT = data.tile([128, PT_LEN], bf16)
        # out-proj partial sums over head-pairs 0..2 for t-tiles 0..7,
        # accumulated as PE filler during heads 6-7 (pairs 0-2 are final then)
        os_part = data.tile([128, 8, 2, 512], bf16)

        nc.gpsimd.memset(v2_sb[:, :, :, 0:64], 1.0)

        # DMA: priority order so compute can start early (first PE work is
        # qk_proj(0) n=0 which needs only wq/wk m=0 + x t-block 0)
        wqk = ctx.enter_context(tc.tile_pool(name="wqk", bufs=2))
        wm_tiles = {}

        def dma_wqk(m):
            for w_dram, tg in ((wq, "wq"), (wk, "wk")):
                wm = wqk.tile([128, KO, 128], bf16, name=f"{tg}{m}", tag=tg)
                nc.sync.dma_start(
                    wm[:],
                    w_dram.rearrange("(ko p) d -> p ko d", p=128)[
                        :, :, 128 * m : 128 * m + 128
                    ],
                )
                wm_tiles[(m, tg)] = wm

        dma_wqk(0)
        xTr = xT.rearrange("(ko p) t -> p ko t", p=128)
        for k in range(KO):
            nc.sync.dma_start(xT_sb[:, k, 0:512], xTr[:, k, 0:512])
        nc.sync.dma_start(wv_sb[:], wv.rearrange("(ko p) d -> p ko d", p=128))
        for tb in range(1, 4):
            for k in range(KO):
                nc.sync.dma_start(
                    xT_sb[:, k, 512 * tb : 512 * tb + 512],
                    xTr[:, k, 512 * tb : 512 * tb + 512],
                )
        nc.sync.dma_start(wp_sb[:], wp.rearrange("(m p) e -> p m e", p=128))

        pj = ctx.enter_context(tc.tile_pool(name="pj", bufs=2, space="PSUM"))
        sT_ps = ctx.enter_context(tc.tile_pool(name="sT_ps", bufs=2, space="PSUM"))
        yq_ps = ctx.enter_context(tc.tile_pool(name="yq_ps", bufs=2, space="PSUM"))

        norm = ctx.enter_context(tc.tile_pool(name="norm", bufs=2))
        ost = ctx.enter_context(tc.tile_pool(name="ost", bufs=2))

        def v_proj(mt):
            ps = pj.tile([128, 512], f32, name=f"psv{mt}", tag="pj")
            for k in range(KO):
                nc.tensor.matmul(
                    ps[:],
                    lhsT=xT_sb[:, k, 128 * mt : 128 * mt + 128],
                    rhs=wv_sb[:, k],
                    start=(k == 0),
                    stop=(k == KO - 1),
                )
            # scatter per-head 64-col blocks into v2 (even l -> cols 0:64,
            # odd l -> cols 64:128)
            psv = ps.rearrange("p (l c) -> p l c", c=64)
            nc.vector.tensor_copy(v2_sb[:, mt, :, 64:128], psv[:])

        def qk_group(m, tg, n):
            wm = wm_tiles[(m, tg)]
            o_sb = qT_sb if tg == "wq" else kT_sb
            ps = pj.tile([128, 512], f32, name=f"psqk{tg}{m}{n}", tag="pj")
            for k in range(KO):
                nc.tensor.matmul(
                    ps[:],
                    lhsT=wm[:, k, :],
                    rhs=xT_sb[:, k, 512 * n : 512 * n + 512],
                    start=(k == 0),
                    stop=(k == KO - 1),
                )
            nc.vector.tensor_copy(o_sb[:, m, 512 * n : 512 * n + 512], ps[:])

        def score_tile(h, i):
            hm, hp = h // 2, h % 2
            pb = 64 * hp
            off = _off(i)
            W = T - 128 * i
            for c0 in range(0, W, 1024):
                Wc = min(1024, W - c0)
                st = sT_ps.tile([128, 1024], f32, name=f"st{h}_{i}_{c0}", tag="sT")
                for s0 in range(0, Wc, 512):
                    Ws = min(512, Wc - s0)
                    nc.tensor.matmul(
                        st[:, s0 : s0 + Ws],
                        lhsT=kT_sb[pb : pb + 64, hm, 128 * i : 128 * i + 128],
                        rhs=qT_sb[pb : pb + 64, hm, 128 * i + c0 + s0 :][:, :Ws],
                        start=True,
                        stop=True,
                    )
                nc.scalar.activation(
                    pT[:, off + c0 : off + c0 + Wc], st[:, :Wc], EXP, scale=0.125
                )
                if c0 == 0:
                    # causal mask inside the diagonal 128x128 block
                    nc.vector.tensor_mul(
                        pT[:, off : off + 128], pT[:, off : off + 128], mask_sb[:]
                    )

        # filler queue: qk projections for m=1..3, one (w, n) group per slot
        filler = [(m, tg, n) for m in (1, 2, 3) for n in range(4) for tg in ("wq", "wk")]
        fi = 0

        def out_stage(mt, n):
            # partial out-proj: pairs m=0..2 -> bf16 staging (pair 3 + add later)
            ps = pj.tile([128, 512], f32, name=f"pst{mt}{n}", tag="pj")
            for m in range(3):
                nc.tensor.matmul(
                    ps[:],
                    lhsT=yTn_sb[:, m, 128 * mt : 128 * mt + 128],
                    rhs=wp_sb[:, m, 512 * n : 512 * n + 512],
                    start=(m == 0),
                    stop=(m == 2),
                )
            nc.vector.tensor_copy(os_part[:, mt, n, :], ps[:])

        stage_q = [(mt, n) for mt in range(8) for n in range(2)]
        si = 0

        # ---- phase 1: v proj + qk proj m=0, per t-block as DMA lands ----
        for tb in range(4):
            qk_group(0, "wq", tb)
            qk_group(0, "wk", tb)
            if tb < 2:
                for mt in range(4 * tb, 4 * tb + 4):
                    v_proj(mt)

        # ---- scores for head 0, v proj mt 8..15 interleaved as filler ----
        defer_v = list(range(8, 16))
        for i in range(16):
            score_tile(0, i)
            if i % 2 == 0 and defer_v:
                v_proj(defer_v.pop(0))

        outr = out.rearrange("(mt p) e -> p mt e", p=128)

        # ---- attention heads ----
        for h in range(NHC):
            m = h // 2
            if h % 2 == 0 and m + 1 < 4:
                dma_wqk(m + 1)
            for q in range(4):
                # PE filler (also: scores h+1 below read qT/kT of pair m+1,
                # so every m+1 group must be emitted before h's q3 i-loop)
                if fi < len(filler) and h < 6:
                    qk_group(*filler[fi])
                    fi += 1
                elif h >= 6:
                    for _ in range(2):
                        if si < len(stage_q):
                            out_stage(*stage_q[si])
                            si += 1
                ilim = min(4 * q + 4, 16)
                yq = yq_ps.tile([128, 512], f32, name=f"yq{h}{q}", tag="yq")
                for i in range(ilim):
                    c0 = max(0, 128 * i - 512 * q)  # col offset inside quarter
                    w = 512 - c0
                    po = _off(i) + 512 * q + c0 - 128 * i
                    nc.tensor.matmul(
                        yq[:, c0:512],
                        lhsT=v2_sb[:, i, h, :],
                        rhs=pT[:, po : po + w],
                        start=(i == 0),
                        stop=(i == ilim - 1),
                    )
                    if q == 3 and h < NHC - 1:
                        score_tile(h + 1, i)
                # normalize: y rows 64:128, rowsum replicated at rows 0:64
                ri = norm.tile([128, 512], f32, name=f"ri{h}{q}", tag="ri")
                sc = norm.tile([128, 512], f32, name=f"sc{h}{q}", tag="sc")
                rib = norm.tile([128, 512], bf16, name=f"rib{h}{q}", tag="rib")
                rbs = norm.tile([128, 512], f32, name=f"rbs{h}{q}", tag="rbs")
                nc.vector.reciprocal_approx_accurate(
                    out=ri[0:64, :], in_=yq[0:64, :], scratch=sc[0:64, :]
                )
                nc.vector.tensor_copy(rib[0:1, :], ri[0:1, :])
                # broadcast the reciprocal row to all partitions via a K=1
                # bf16 matmul (ones column x recip row), staged through PSUM
                rb_ps = pj.tile([128, 512], f32, name=f"rbp{h}{q}", tag="pj")
                nc.tensor.matmul(
                    rb_ps[:],
                    lhsT=ones_bf[0:1, :],
                    rhs=rib[0:1, :],
                    start=True,
                    stop=True,
                )
                nc.vector.tensor_copy(rbs[:], rb_ps[:])
                cols = slice(512 * q, 512 * q + 512)
                if h % 2 == 1:
                    nc.vector.tensor_mul(
                        yTn_sb[64:128, m, cols], yq[64:128, :], rbs[64:128, :]
                    )
                else:
                    # even heads land at partitions 0:64 of the paired layout;
                    # engines can't shift partitions, so stage + SBUF DMA
                    yt_h = norm.tile([128, 512], bf16, name=f"ytmp{h}{q}", tag="ytmp")
                    nc.vector.tensor_mul(
                        yt_h[64:128, :], yq[64:128, :], rbs[64:128, :]
                    )
                    nc.sync.dma_start(yTn_sb[0:64, m, cols], yt_h[64:128, :])

        # ---- out projection: out[t, e] = y @ Wp_sub^T, head-paired K=128.
        # t-tiles 0..7: pair-3 matmul + staged partials; 8..15: all 4 pairs.
        # psum tiles cycle through all three psum pools (attention is done) for
        # a 6-deep rotation — a 2-slot rotation made the tail latency-bound.
        # psum->sbuf copies alternate scalar/vector so neither engine binds.
        pool_cyc = [(pj, "pj"), (sT_ps, "sT"), (yq_ps, "yq")]
        for mt in range(TM):
            for n in range(2):
                pool, ptag = pool_cyc[(2 * mt + n) % 3]
                ps = pool.tile([128, 512], f32, name=f"pso{mt}{n}", tag=ptag)
                m0 = 3 if mt < 8 else 0
                for m in range(m0, 4):
                    nc.tensor.matmul(
                        ps[:],
                        lhsT=yTn_sb[:, m, 128 * mt : 128 * mt + 128],
                        rhs=wp_sb[:, m, 512 * n : 512 * n + 512],
                        start=(m == m0),
                        stop=(m == 3),
                    )
                o_sb = ost.tile([128, 512], f32, name=f"ost{mt}{n}", tag="ost")
                if mt < 8:
                    nc.vector.tensor_add(o_sb[:], ps[:], os_part[:, mt, n, :])
                elif (2 * mt + n) % 2:
                    nc.vector.tensor_copy(o_sb[:], ps[:])
                else:
                    nc.scalar.copy(o_sb[:], ps[:])
                nc.sync.dma_start(outr[:, mt, 512 * n : 512 * n + 512], o_sb[:])

    nc.compile()
    return nc


def _get_nc():
    global _NC
    if _NC is None:
        _NC = build_nc()
    return _NC


def kernel(x, Wk, Wq, Wv, Wp, _trace=False):
    from concourse.bass_utils import run_bass_kernel_spmd

    global LAST_RESULTS
    bf16 = ml_dtypes.bfloat16
    x = np.asarray(x, dtype=np.float32)
    Wk = np.asarray(Wk, dtype=np.float32)
    Wq = np.asarray(Wq, dtype=np.float32)
    Wv = np.asarray(Wv, dtype=np.float32)
    Wp = np.asarray(Wp, dtype=np.float32)

    in_maps = []
    for c in range(8):
        b, hh = c // 2, c % 2
        cols = slice(CC * hh, CC * hh + CC)
        in_maps.append(
            {
                "xT": np.ascontiguousarray(x[b].T).astype(bf16),
                "wqT": np.ascontiguousarray(Wq[cols, :].T).astype(bf16),
                "wkT": np.ascontiguousarray(Wk[cols, :].T).astype(bf16),
                "wvT": np.ascontiguousarray(Wv[cols, :].T).astype(bf16),
                "wpT": np.ascontiguousarray(Wp[:, cols].T).astype(bf16),
            }
        )

    nc = _get_nc()
    res = run_bass_kernel_spmd(nc, in_maps, core_ids=list(range(8)), trace=_trace)
    LAST_RESULTS = res

    out = np.empty((B, T, C), dtype=np.float32)
    for b in range(B):
        out[b] = res.results[2 * b]["out"] + res.results[2 * b + 1]["out"]
    return out


if __name__ == "__main__":
    rng = np.random.default_rng(0)
    s = 1.0 / np.sqrt(C)
    inputs = {
        "x": rng.standard_normal((B, T, C), dtype=np.float32),
        "Wk": rng.standard_normal((C, C), dtype=np.float32) * s,
        "Wq": rng.standard_normal((C, C), dtype=np.float32) * s,
        "Wv": rng.standard_normal((C, C), dtype=np.float32) * s,
        "Wp": rng.standard_normal((C, C), dtype=np.float32) * s,
    }
    got = kernel(**inputs)

    def ref(x, Wk, Wq, Wv, Wp):
        def heads(w):
            return (
                np.einsum("btc,ec->bte", x, w)
                .reshape(B, T, N_HEAD, D)
                .transpose(0, 2, 1, 3)
            )

        k, q, v = heads(Wk), heads(Wq), heads(Wv)
        att = np.einsum("bhqd,bhkd->bhqk", q, k) / np.sqrt(D)
        causal = np.tril(np.ones((T, T), dtype=bool))
        att = np.where(causal[None, None], att, -np.inf)
        att = att - att.max(axis=-1, keepdims=True)
        e = np.exp(att)
        p = e / e.sum(axis=-1, keepdims=True)
        y = np.einsum("bhqk,bhkd->bhqd", p, v)
        y = y.transpose(0, 2, 1, 3).reshape(B, T, C)
        return np.einsum("btc,ec->bte", y, Wp)

    want = ref(**{k: v.astype(np.float64) for k, v in inputs.items()}).astype(
        np.float32
    )
    rel = np.linalg.norm(got - want) / np.linalg.norm(want)
    print("rel l2 err:", rel)
    print("max abs err:", np.abs(got - want).max(), "ref absmax:", np.abs(want).max())


# revision 16
# speedup vs baseline: 1.0799x; 1.0538x over previous
"""Causal self-attention (B=4, T=2048, C=1024, 16 heads) on 8 trn2 NeuronCores.

Sharding: core c -> (batch b = c//2, head-half hh = c%2). Each core computes
one batch x 8 heads: QKV column-parallel + out-proj row-parallel (Megatron);
the host sums the two partial outputs per batch. No collectives.

Schedule (single continuous PE stream to keep the HAM clock at 2.4 GHz):
  - DMA ordered so v/qk projections start as soon as the first x t-block lands
  - exp table preloaded at t~0 via a dummy activation
  - attention per head: scores (transposed, causal-trimmed) -> exp on ACT ->
    y accumulated in 512-col "quarters"; next head's scores interleave into
    the last quarter's tk passes so the single pT buffer recycles per region
  - qk projections for later head-pairs are emitted as PE filler inside the
    attention phase (covers the ACT exp deficit)
  - out-projection pairs heads (2m, 2m+1) into a K=128 contraction
  - all matmuls bf16 (1 cycle/row); rowsum-reciprocal broadcast via a K=1
    bf16 matmul

Self-contained: hardcodes shapes; builds/compiles the Bass program once per
process and runs it via run_bass_kernel_spmd on cores 0-7.
"""

import numpy as np
import ml_dtypes

B, T, C = 4, 2048, 1024
N_HEAD = 16
D = 64          # head dim
NHC = 8         # heads per core
CC = 512        # channels per core (NHC * D)
KO = 8          # contraction chunks of 128 over C
TM = 16         # t chunks of 128

_NC = None          # cached compiled Bass program
LAST_RESULTS = None  # BassKernelResults of the last run (for test harness)


def _off(i):
    # start offset of score tile i inside the packed pT tensor
    return 2048 * i - 64 * i * (i - 1)


PT_LEN = _off(16)  # 17408


def build_nc():
    import concourse.bacc as bacc
    import concourse.mybir as mybir
    import concourse.tile as tile
    from concourse.masks import make_identity, make_upper_triangular
    from contextlib import ExitStack

    bf16 = mybir.dt.bfloat16
    f32 = mybir.dt.float32
    EXP = mybir.ActivationFunctionType.Exp

    nc = bacc.Bacc("TRN2", target_bir_lowering=False, debug=False)

    xT = nc.dram_tensor("xT", [C, T], bf16, kind="ExternalInput")
    wq = nc.dram_tensor("wqT", [C, CC], bf16, kind="ExternalInput")
    wk = nc.dram_tensor("wkT", [C, CC], bf16, kind="ExternalInput")
    wv = nc.dram_tensor("wvT", [C, CC], bf16, kind="ExternalInput")
    wp = nc.dram_tensor("wpT", [CC, C], bf16, kind="ExternalInput")
    out = nc.dram_tensor("out", [T, C], f32, kind="ExternalOutput")

    with tile.TileContext(nc) as tc, ExitStack() as ctx:
        const = ctx.enter_context(tc.tile_pool(name="const", bufs=1))
        warm_i = const.tile([128, 8], f32)
        warm_o = const.tile([128, 8], f32)
        mask_sb = const.tile([128, 128], bf16)
        ones_bf = const.tile([128, 128], bf16)
        ident_sb = const.tile([128, 128], bf16)

        # pull the exp table load to t~0 (it costs ~2.7us on first use)
        nc.gpsimd.memset(warm_i[0:1, :], 0.0)
        nc.scalar.activation(warm_o[0:1, :], warm_i[0:1, :], EXP)
        # keep-mask for the diagonal 128x128 block of p.T tiles: 1 where tq>=tk
        make_upper_triangular(nc, mask_sb[:], val=1.0, diag=True)
        nc.gpsimd.memset(ones_bf[0:1, :], 1.0)
        make_identity(nc, ident_sb[:])

        wts = ctx.enter_context(tc.tile_pool(name="wts", bufs=1))
        xT_sb = wts.tile([128, KO, T], bf16)
        wv_sb = wts.tile([128, KO, CC], bf16)
        wp_sb = wts.tile([128, 4, C], bf16)

        data = ctx.enter_context(tc.tile_pool(name="data", bufs=1))
        qT_sb = data.tile([128, 4, T], bf16)
        kT_sb = data.tile([128, 4, T], bf16)
        # v2[p, ti, l, 0:128] = [ones(64) | v_l(64)] for every head l: the y
        # matmul then yields the softmax denominator at psum rows 0:64 and y
        # at rows 64:128 in one pass
        v2_sb = data.tile([128, TM, NHC, 128], bf16)
        # normalized y, head-paired: head h lives at partitions 64*(h%2) in
        # slot m=h//2, giving a K=128 contraction for the out-projection
        yTn_sb = data.tile([128, 4, T], bf16)
        pT = data.tile([128, PT_LEN], bf16)
        # out-proj partial sums over head-pairs 0..2 for t-tiles 0..7,
        # accumulated as PE filler during heads 6-7 (pairs 0-2 are final then)
        os_part = data.tile([128, 8, 2, 512], bf16)

        nc.gpsimd.memset(v2_sb[:, :, :, 0:64], 1.0)

        # DMA: priority order so compute can start early (first PE work is
        # qk_proj(0) n=0 which needs only wq/wk m=0 + x t-block 0)
        wqk = ctx.enter_context(tc.tile_pool(name="wqk", bufs=2))
        wm_tiles = {}

        def dma_wqk(m):
            for w_dram, tg in ((wq, "wq"), (wk, "wk")):
                wm = wqk.tile([128, KO, 128], bf16, name=f"{tg}{m}", tag=tg)
                nc.sync.dma_start(
                    wm[:],
                    w_dram.rearrange("(ko p) d -> p ko d", p=128)[
                        :, :, 128 * m : 128 * m + 128
                    ],
                )
                wm_tiles[(m, tg)] = wm

        dma_wqk(0)
        xTr = xT.rearrange("(ko p) t -> p ko t", p=128)
        for k in range(KO):
            nc.sync.dma_start(xT_sb[:, k, 0:512], xTr[:, k, 0:512])
        nc.sync.dma_start(wv_sb[:], wv.rearrange("(ko p) d -> p ko d", p=128))
        for tb in range(1, 4):
            for k in range(KO):
                nc.sync.dma_start(
                    xT_sb[:, k, 512 * tb : 512 * tb + 512],
                    xTr[:, k, 512 * tb : 512 * tb + 512],
                )
        nc.sync.dma_start(wp_sb[:], wp.rearrange("(m p) e -> p m e", p=128))

        pj = ctx.enter_context(tc.tile_pool(name="pj", bufs=2, space="PSUM"))
        sT_ps = ctx.enter_context(tc.tile_pool(name="sT_ps", bufs=2, space="PSUM"))
        yq_ps = ctx.enter_context(tc.tile_pool(name="yq_ps", bufs=2, space="PSUM"))

        norm = ctx.enter_context(tc.tile_pool(name="norm", bufs=2))
        ost = ctx.enter_context(tc.tile_pool(name="ost", bufs=4))

        def v_proj(mt):
            ps = pj.tile([128, 512], f32, name=f"psv{mt}", tag="pj")
            for k in range(KO):
                nc.tensor.matmul(
                    ps[:],
                    lhsT=xT_sb[:, k, 128 * mt : 128 * mt + 128],
                    rhs=wv_sb[:, k],
                    start=(k == 0),
                    stop=(k == KO - 1),
                )
            # scatter per-head 64-col blocks into v2 (even l -> cols 0:64,
            # odd l -> cols 64:128)
            psv = ps.rearrange("p (l c) -> p l c", c=64)
            nc.vector.tensor_copy(v2_sb[:, mt, :, 64:128], psv[:])

        def qk_group(m, tg, n):
            wm = wm_tiles[(m, tg)]
            o_sb = qT_sb if tg == "wq" else kT_sb
            ps = pj.tile([128, 512], f32, name=f"psqk{tg}{m}{n}", tag="pj")
            for k in range(KO):
                nc.tensor.matmul(
                    ps[:],
                    lhsT=wm[:, k, :],
                    rhs=xT_sb[:, k, 512 * n : 512 * n + 512],
                    start=(k == 0),
                    stop=(k == KO - 1),
                )
            nc.vector.tensor_copy(o_sb[:, m, 512 * n : 512 * n + 512], ps[:])

        def score_tile(h, i):
            hm, hp = h // 2, h % 2
            pb = 64 * hp
            off = _off(i)
            W = T - 128 * i
            for c0 in range(0, W, 1024):
                Wc = min(1024, W - c0)
                st = sT_ps.tile([128, 1024], f32, name=f"st{h}_{i}_{c0}", tag="sT")
                for s0 in range(0, Wc, 512):
                    Ws = min(512, Wc - s0)
                    nc.tensor.matmul(
                        st[:, s0 : s0 + Ws],
                        lhsT=kT_sb[pb : pb + 64, hm, 128 * i : 128 * i + 128],
                        rhs=qT_sb[pb : pb + 64, hm, 128 * i + c0 + s0 :][:, :Ws],
                        start=True,
                        stop=True,
                    )
                nc.scalar.activation(
                    pT[:, off + c0 : off + c0 + Wc], st[:, :Wc], EXP, scale=0.125
                )
                if c0 == 0:
                    # causal mask inside the diagonal 128x128 block
                    nc.vector.tensor_mul(
                        pT[:, off : off + 128], pT[:, off : off + 128], mask_sb[:]
                    )

        # filler queue: qk projections for m=1..3, one (w, n) group per slot
        filler = [(m, tg, n) for m in (1, 2, 3) for n in range(4) for tg in ("wq", "wk")]
        fi = 0

        def out_stage(mt, n):
            # partial out-proj: pairs m=0..2 -> bf16 staging (pair 3 + add later)
            ps = pj.tile([128, 512], f32, name=f"pst{mt}{n}", tag="pj")
            for m in range(3):
                nc.tensor.matmul(
                    ps[:],
                    lhsT=yTn_sb[:, m, 128 * mt : 128 * mt + 128],
                    rhs=wp_sb[:, m, 512 * n : 512 * n + 512],
                    start=(m == 0),
                    stop=(m == 2),
                )
            nc.vector.tensor_copy(os_part[:, mt, n, :], ps[:])

        stage_q = [(mt, n) for mt in range(8) for n in range(2)]
        si = 0

        # ---- phase 1: v proj + qk proj m=0, per t-block as DMA lands ----
        for tb in range(4):
            qk_group(0, "wq", tb)
            qk_group(0, "wk", tb)
            if tb < 2:
                for mt in range(4 * tb, 4 * tb + 4):
                    v_proj(mt)

        # ---- scores for head 0, v proj mt 8..15 interleaved as filler ----
        defer_v = list(range(8, 16))
        for i in range(16):
            score_tile(0, i)
            if i % 2 == 0 and defer_v:
                v_proj(defer_v.pop(0))

        outr = out.rearrange("(mt p) e -> p mt e", p=128)

        # ---- attention heads ----
        for h in range(NHC):
            m = h // 2
            if h % 2 == 0 and m + 1 < 4:
                dma_wqk(m + 1)
            for q in range(4):
                # PE filler (also: scores h+1 below read qT/kT of pair m+1,
                # so every m+1 group must be emitted before h's q3 i-loop)
                if fi < len(filler) and h < 6:
                    qk_group(*filler[fi])
                    fi += 1
                elif h >= 6:
                    for _ in range(2):
                        if si < len(stage_q):
                            out_stage(*stage_q[si])
                            si += 1
                ilim = min(4 * q + 4, 16)
                yq = yq_ps.tile([128, 512], f32, name=f"yq{h}{q}", tag="yq")
                for i in range(ilim):
                    c0 = max(0, 128 * i - 512 * q)  # col offset inside quarter
                    w = 512 - c0
                    po = _off(i) + 512 * q + c0 - 128 * i
                    nc.tensor.matmul(
                        yq[:, c0:512],
                        lhsT=v2_sb[:, i, h, :],
                        rhs=pT[:, po : po + w],
                        start=(i == 0),
                        stop=(i == ilim - 1),
                    )
                    if q == 3 and h < NHC - 1:
                        score_tile(h + 1, i)
                # normalize: y rows 64:128, rowsum replicated at rows 0:64
                ri = norm.tile([128, 512], f32, name=f"ri{h}{q}", tag="ri")
                rib = norm.tile([128, 512], bf16, name=f"rib{h}{q}", tag="rib")
                # rbs doubles as recip scratch: both uses are DVE-sequential
                # (scratch write, then the rb_ps copy overwrites it)
                rbs = norm.tile([128, 512], f32, name=f"rbs{h}{q}", tag="rbs")
                nc.vector.reciprocal_approx_accurate(
                    out=ri[0:64, :], in_=yq[0:64, :], scratch=rbs[0:64, :]
                )
                nc.vector.tensor_copy(rib[0:1, :], ri[0:1, :])
                # broadcast the reciprocal row to all partitions via a K=1
                # bf16 matmul (ones column x recip row), staged through PSUM
                rb_ps = pj.tile([128, 512], f32, name=f"rbp{h}{q}", tag="pj")
                nc.tensor.matmul(
                    rb_ps[:],
                    lhsT=ones_bf[0:1, :],
                    rhs=rib[0:1, :],
                    start=True,
                    stop=True,
                )
                nc.vector.tensor_copy(rbs[:], rb_ps[:])
                cols = slice(512 * q, 512 * q + 512)
                if h % 2 == 1:
                    nc.vector.tensor_mul(
                        yTn_sb[64:128, m, cols], yq[64:128, :], rbs[64:128, :]
                    )
                else:
                    # even heads land at partitions 0:64 of the paired layout;
                    # engines can't shift partitions, so stage + SBUF DMA
                    yt_h = norm.tile([128, 512], bf16, name=f"ytmp{h}{q}", tag="ytmp")
                    nc.vector.tensor_mul(
                        yt_h[64:128, :], yq[64:128, :], rbs[64:128, :]
                    )
                    nc.sync.dma_start(yTn_sb[0:64, m, cols], yt_h[64:128, :])

        # ---- out projection: out[t, e] = y @ Wp_sub^T, head-paired K=128.
        # t-tiles 0..7: pair-3 matmul + staged partials; 8..15: all 4 pairs.
        # psum tiles cycle through all three psum pools (attention is done) for
        # a 6-deep rotation — a 2-slot rotation made the tail latency-bound.
        # psum->sbuf copies alternate scalar/vector so neither engine binds.
        pool_cyc = [(pj, "pj"), (sT_ps, "sT"), (yq_ps, "yq")]
        for mt in range(TM):
            for n in range(2):
                pool, ptag = pool_cyc[(2 * mt + n) % 3]
                ps = pool.tile([128, 512], f32, name=f"pso{mt}{n}", tag=ptag)
                if mt < 8:
                    # fold the bf16 staged partial into psum via an identity
                    # matmul (keeps the tail PE-dense; no cross-engine add)
                    nc.tensor.matmul(
                        ps[:],
                        lhsT=ident_sb[:],
                        rhs=os_part[:, mt, n, :],
                        start=True,
                        stop=False,
                    )
                m0 = 3 if mt < 8 else 0
                for m in range(m0, 4):
                    nc.tensor.matmul(
                        ps[:],
                        lhsT=yTn_sb[:, m, 128 * mt : 128 * mt + 128],
                        rhs=wp_sb[:, m, 512 * n : 512 * n + 512],
                        start=(m == m0) and mt >= 8,
                        stop=(m == 3),
                    )
                o_sb = ost.tile([128, 512], f32, name=f"ost{mt}{n}", tag="ost")
                if (2 * mt + n) % 2:
                    nc.vector.tensor_copy(o_sb[:], ps[:])
                else:
                    nc.scalar.copy(o_sb[:], ps[:])
                nc.sync.dma_start(outr[:, mt, 512 * n : 512 * n + 512], o_sb[:])

    nc.compile()
    return nc


def _get_nc():
    global _NC
    if _NC is None:
        _NC = build_nc()
    return _NC


def kernel(x, Wk, Wq, Wv, Wp, _trace=False):
    from concourse.bass_utils import run_bass_kernel_spmd

    global LAST_RESULTS
    bf16 = ml_dtypes.bfloat16
    x = np.asarray(x, dtype=np.float32)
    Wk = np.asarray(Wk, dtype=np.float32)
    Wq = np.asarray(Wq, dtype=np.float32)
    Wv = np.asarray(Wv, dtype=np.float32)
    Wp = np.asarray(Wp, dtype=np.float32)

    in_maps = []
    for c in range(8):
        b, hh = c // 2, c % 2
        cols = slice(CC * hh, CC * hh + CC)
        in_maps.append(
            {
                "xT": np.ascontiguousarray(x[b].T).astype(bf16),
                "wqT": np.ascontiguousarray(Wq[cols, :].T).astype(bf16),
                "wkT": np.ascontiguousarray(Wk[cols, :].T).astype(bf16),
                "wvT": np.ascontiguousarray(Wv[cols, :].T).astype(bf16),
                "wpT": np.ascontiguousarray(Wp[:, cols].T).astype(bf16),
            }
        )

    nc = _get_nc()
    res = run_bass_kernel_spmd(nc, in_maps, core_ids=list(range(8)), trace=_trace)
    LAST_RESULTS = res

    out = np.empty((B, T, C), dtype=np.float32)
    for b in range(B):
        out[b] = res.results[2 * b]["out"] + res.results[2 * b + 1]["out"]
    return out


if __name__ == "__main__":
    rng = np.random.default_rng(0)
    s = 1.0 / np.sqrt(C)
    inputs = {
        "x": rng.standard_normal((B, T, C), dtype=np.float32),
        "Wk": rng.standard_normal((C, C), dtype=np.float32) * s,
        "Wq": rng.standard_normal((C, C), dtype=np.float32) * s,
        "Wv": rng.standard_normal((C, C), dtype=np.float32) * s,
        "Wp": rng.standard_normal((C, C), dtype=np.float32) * s,
    }
    got = kernel(**inputs)

    def ref(x, Wk, Wq, Wv, Wp):
        def heads(w):
            return (
                np.einsum("btc,ec->bte", x, w)
                .reshape(B, T, N_HEAD, D)
                .transpose(0, 2, 1, 3)
            )

        k, q, v = heads(Wk), heads(Wq), heads(Wv)
        att = np.einsum("bhqd,bhkd->bhqk", q, k) / np.sqrt(D)
        causal = np.tril(np.ones((T, T), dtype=bool))
        att = np.where(causal[None, None], att, -np.inf)
        att = att - att.max(axis=-1, keepdims=True)
        e = np.exp(att)
        p = e / e.sum(axis=-1, keepdims=True)
        y = np.einsum("bhqk,bhkd->bhqd", p, v)
        y = y.transpose(0, 2, 1, 3).reshape(B, T, C)
        return np.einsum("btc,ec->bte", y, Wp)

    want = ref(**{k: v.astype(np.float64) for k, v in inputs.items()}).astype(
        np.float32
    )
    rel = np.linalg.norm(got - want) / np.linalg.norm(want)
    print("rel l2 err:", rel)
    print("max abs err:", np.abs(got - want).max(), "ref absmax:", np.abs(want).max())


# revision 24
# speedup vs baseline: 1.0974x; 1.0162x over previous
"""Causal self-attention (B=4, T=2048, C=1024, 16 heads) on 8 trn2 NeuronCores.

Sharding: core c -> (batch b = c//2, head-half hh = c%2). Each core computes
one batch x 8 heads: QKV column-parallel + out-proj row-parallel (Megatron);
the host sums the two partial outputs per batch. No collectives.

Schedule (single continuous PE stream to keep the HAM clock at 2.4 GHz):
  - DMA ordered so v/qk projections start as soon as the first x t-block lands
  - exp table preloaded at t~0 via a dummy activation
  - attention per head: scores (transposed, causal-trimmed) -> exp on ACT ->
    y accumulated in 512-col "quarters"; next head's scores interleave into
    the last quarter's tk passes so the single pT buffer recycles per region
  - qk projections for later head-pairs are emitted as PE filler inside the
    attention phase (covers the ACT exp deficit)
  - out-projection pairs heads (2m, 2m+1) into a K=128 contraction
  - all matmuls bf16 (1 cycle/row); rowsum-reciprocal broadcast via a K=1
    bf16 matmul

Self-contained: hardcodes shapes; builds/compiles the Bass program once per
process and runs it via run_bass_kernel_spmd on cores 0-7.
"""

import numpy as np
import ml_dtypes

B, T, C = 4, 2048, 1024
N_HEAD = 16
D = 64          # head dim
NHC = 8         # heads per core
CC = 512        # channels per core (NHC * D)
KO = 8          # contraction chunks of 128 over C
TM = 16         # t chunks of 128

_NC = None          # cached compiled Bass program
LAST_RESULTS = None  # BassKernelResults of the last run (for test harness)


def _off(i):
    # start offset of score tile i inside the packed pT tensor
    return 2048 * i - 64 * i * (i - 1)


PT_LEN = _off(16)  # 17408


def build_nc():
    import concourse.bacc as bacc
    import concourse.mybir as mybir
    import concourse.tile as tile
    from concourse.masks import make_identity, make_upper_triangular
    from contextlib import ExitStack

    bf16 = mybir.dt.bfloat16
    f32 = mybir.dt.float32
    EXP = mybir.ActivationFunctionType.Exp

    nc = bacc.Bacc("TRN2", target_bir_lowering=False, debug=False)

    xT = nc.dram_tensor("xT", [C, T], bf16, kind="ExternalInput")
    wq = nc.dram_tensor("wqT", [C, CC], bf16, kind="ExternalInput")
    wk = nc.dram_tensor("wkT", [C, CC], bf16, kind="ExternalInput")
    wv = nc.dram_tensor("wvT", [C, CC], bf16, kind="ExternalInput")
    wp = nc.dram_tensor("wpT", [CC, C], bf16, kind="ExternalInput")
    out = nc.dram_tensor("out", [T, C], f32, kind="ExternalOutput")

    with tile.TileContext(nc) as tc, ExitStack() as ctx:
        const = ctx.enter_context(tc.tile_pool(name="const", bufs=1))
        warm_i = const.tile([128, 8], f32)
        warm_o = const.tile([128, 8], f32)
        mask_sb = const.tile([128, 128], bf16)
        ones_bf = const.tile([128, 128], bf16)
        ident_sb = const.tile([128, 128], bf16)

        # pull the exp table load to t~0 (it costs ~2.7us on first use)
        nc.gpsimd.memset(warm_i[0:1, :], 0.0)
        nc.scalar.activation(warm_o[0:1, :], warm_i[0:1, :], EXP)
        # keep-mask for the diagonal 128x128 block of p.T tiles: 1 where tq>=tk
        make_upper_triangular(nc, mask_sb[:], val=1.0, diag=True)
        nc.gpsimd.memset(ones_bf[0:1, :], 1.0)
        make_identity(nc, ident_sb[:])

        wts = ctx.enter_context(tc.tile_pool(name="wts", bufs=1))
        xT_sb = wts.tile([128, KO, T], bf16)
        wv_sb = wts.tile([128, KO, CC], bf16)
        wq_sb = wts.tile([128, KO, CC], bf16)
        wk_sb = wts.tile([128, KO, CC], bf16)
        wp_sb = wts.tile([128, 4, C], bf16)

        data = ctx.enter_context(tc.tile_pool(name="data", bufs=1))
        qT_sb = data.tile([128, 4, T], bf16)
        kT_sb = data.tile([128, 4, T], bf16)
        # v2[p, ti, l, 0:128] = [ones(64) | v_l(64)] for every head l: the y
        # matmul then yields the softmax denominator at psum rows 0:64 and y
        # at rows 64:128 in one pass
        v2_sb = data.tile([128, TM, NHC, 128], bf16)
        # normalized y, head-paired: head h lives at partitions 64*(h%2) in
        # slot m=h//2, giving a K=128 contraction for the out-projection
        yTn_sb = data.tile([128, 4, T], bf16)
        pT = data.tile([128, PT_LEN], bf16)
        # out-proj partial sums over head-pairs 0..2 for t-tiles 0..3,
        # accumulated as PE filler during heads 6-7 (pairs 0-2 are final then)
        os_part = data.tile([128, 4, 2, 512], bf16)

        nc.gpsimd.memset(v2_sb[:, :, :, 0:64], 1.0)

        # DMA: priority order so compute can start early (first PE work is
        # qk_proj(0) n=0 which needs only wq + x t-block 0). wq/wk load
        # full-width once: 1KB contiguous lines, vs 256B-line per-m slices
        # whose packet storms stole SBUF bandwidth mid-kernel.
        nc.sync.dma_start(wq_sb[:], wq.rearrange("(ko p) d -> p ko d", p=128))
        xTr = xT.rearrange("(ko p) t -> p ko t", p=128)
        for k in range(KO):
            nc.sync.dma_start(xT_sb[:, k, 0:512], xTr[:, k, 0:512])
        nc.sync.dma_start(wk_sb[:], wk.rearrange("(ko p) d -> p ko d", p=128))
        nc.sync.dma_start(wv_sb[:], wv.rearrange("(ko p) d -> p ko d", p=128))
        for tb in range(1, 4):
            for k in range(KO):
                nc.sync.dma_start(
                    xT_sb[:, k, 512 * tb : 512 * tb + 512],
                    xTr[:, k, 512 * tb : 512 * tb + 512],
                )
        nc.sync.dma_start(wp_sb[:], wp.rearrange("(m p) e -> p m e", p=128))

        pj = ctx.enter_context(tc.tile_pool(name="pj", bufs=2, space="PSUM"))
        sT_ps = ctx.enter_context(tc.tile_pool(name="sT_ps", bufs=2, space="PSUM"))
        yq_ps = ctx.enter_context(tc.tile_pool(name="yq_ps", bufs=2, space="PSUM"))

        norm = ctx.enter_context(tc.tile_pool(name="norm", bufs=2))
        ost = ctx.enter_context(tc.tile_pool(name="ost", bufs=4))

        def v_proj(mt):
            ps = pj.tile([128, 512], f32, name=f"psv{mt}", tag="pj")
            for k in range(KO):
                nc.tensor.matmul(
                    ps[:],
                    lhsT=xT_sb[:, k, 128 * mt : 128 * mt + 128],
                    rhs=wv_sb[:, k],
                    start=(k == 0),
                    stop=(k == KO - 1),
                )
            # scatter per-head 64-col blocks into v2 (even l -> cols 0:64,
            # odd l -> cols 64:128)
            psv = ps.rearrange("p (l c) -> p l c", c=64)
            nc.vector.tensor_copy(v2_sb[:, mt, :, 64:128], psv[:])

        def qk_group(m, tg, n):
            w_sb = wq_sb if tg == "wq" else wk_sb
            o_sb = qT_sb if tg == "wq" else kT_sb
            ps = pj.tile([128, 512], f32, name=f"psqk{tg}{m}{n}", tag="pj")
            for k in range(KO):
                nc.tensor.matmul(
                    ps[:],
                    lhsT=w_sb[:, k, 128 * m : 128 * m + 128],
                    rhs=xT_sb[:, k, 512 * n : 512 * n + 512],
                    start=(k == 0),
                    stop=(k == KO - 1),
                )
            nc.vector.tensor_copy(o_sb[:, m, 512 * n : 512 * n + 512], ps[:])

        def score_tile(h, i):
            hm, hp = h // 2, h % 2
            pb = 64 * hp
            off = _off(i)
            W = T - 128 * i
            for c0 in range(0, W, 1024):
                Wc = min(1024, W - c0)
                st = sT_ps.tile([128, 1024], f32, name=f"st{h}_{i}_{c0}", tag="sT")
                for s0 in range(0, Wc, 512):
                    Ws = min(512, Wc - s0)
                    nc.tensor.matmul(
                        st[:, s0 : s0 + Ws],
                        lhsT=kT_sb[pb : pb + 64, hm, 128 * i : 128 * i + 128],
                        rhs=qT_sb[pb : pb + 64, hm, 128 * i + c0 + s0 :][:, :Ws],
                        start=True,
                        stop=True,
                    )
                nc.scalar.activation(
                    pT[:, off + c0 : off + c0 + Wc], st[:, :Wc], EXP, scale=0.125
                )
                if c0 == 0:
                    # causal mask inside the diagonal 128x128 block
                    nc.vector.tensor_mul(
                        pT[:, off : off + 128], pT[:, off : off + 128], mask_sb[:]
                    )

        # filler queue: qk projections for m=1..3, one (w, n) group per slot
        filler = [(m, tg, n) for m in (1, 2, 3) for n in range(4) for tg in ("wq", "wk")]
        fi = 0

        def out_stage(mt, n):
            # partial out-proj: pairs m=0..2 -> bf16 staging (pair 3 + add later)
            ps = pj.tile([128, 512], f32, name=f"pst{mt}{n}", tag="pj")
            for m in range(3):
                nc.tensor.matmul(
                    ps[:],
                    lhsT=yTn_sb[:, m, 128 * mt : 128 * mt + 128],
                    rhs=wp_sb[:, m, 512 * n : 512 * n + 512],
                    start=(m == 0),
                    stop=(m == 2),
                )
            nc.vector.tensor_copy(os_part[:, mt, n, :], ps[:])

        stage_q = [(mt, n) for mt in range(4) for n in range(2)]
        si = 0

        # ---- phase 1: v proj + qk proj m=0, per t-block as DMA lands ----
        for tb in range(4):
            qk_group(0, "wq", tb)
            qk_group(0, "wk", tb)
            if tb < 2:
                for mt in range(4 * tb, 4 * tb + 4):
                    v_proj(mt)

        # ---- scores for head 0, v proj mt 8..15 interleaved as filler ----
        defer_v = list(range(8, 16))
        for i in range(16):
            score_tile(0, i)
            if i % 2 == 0 and defer_v:
                v_proj(defer_v.pop(0))

        outr = out.rearrange("(mt p) e -> p mt e", p=128)

        # ---- attention heads ----
        for h in range(NHC):
            m = h // 2
            for q in range(4):
                # PE filler (also: scores h+1 below read qT/kT of pair m+1,
                # so every m+1 group must be emitted before h's q3 i-loop)
                if fi < len(filler) and h < 6:
                    qk_group(*filler[fi])
                    fi += 1
                elif h >= 6 and si < len(stage_q):
                    out_stage(*stage_q[si])
                    si += 1
                ilim = min(4 * q + 4, 16)
                yq = yq_ps.tile([128, 512], f32, name=f"yq{h}{q}", tag="yq")
                for i in range(ilim):
                    c0 = max(0, 128 * i - 512 * q)  # col offset inside quarter
                    w = 512 - c0
                    po = _off(i) + 512 * q + c0 - 128 * i
                    nc.tensor.matmul(
                        yq[:, c0:512],
                        lhsT=v2_sb[:, i, h, :],
                        rhs=pT[:, po : po + w],
                        start=(i == 0),
                        stop=(i == ilim - 1),
                    )
                    if q == 3 and h < NHC - 1:
                        score_tile(h + 1, i)
                # normalize: y rows 64:128, rowsum replicated at rows 0:64
                ri = norm.tile([128, 512], f32, name=f"ri{h}{q}", tag="ri")
                rib = norm.tile([128, 512], bf16, name=f"rib{h}{q}", tag="rib")
                # rbs doubles as recip scratch: both uses are DVE-sequential
                # (scratch write, then the rb_ps copy overwrites it)
                rbs = norm.tile([128, 512], f32, name=f"rbs{h}{q}", tag="rbs")
                nc.vector.reciprocal_approx_accurate(
                    out=ri[0:64, :], in_=yq[0:64, :], scratch=rbs[0:64, :]
                )
                nc.vector.tensor_copy(rib[0:1, :], ri[0:1, :])
                # broadcast the reciprocal row to all partitions via a K=1
                # bf16 matmul (ones column x recip row), staged through PSUM
                rb_ps = pj.tile([128, 512], f32, name=f"rbp{h}{q}", tag="pj")
                nc.tensor.matmul(
                    rb_ps[:],
                    lhsT=ones_bf[0:1, :],
                    rhs=rib[0:1, :],
                    start=True,
                    stop=True,
                )
                nc.vector.tensor_copy(rbs[:], rb_ps[:])
                cols = slice(512 * q, 512 * q + 512)
                if h % 2 == 1:
                    nc.vector.tensor_mul(
                        yTn_sb[64:128, m, cols], yq[64:128, :], rbs[64:128, :]
                    )
                else:
                    # even heads land at partitions 0:64 of the paired layout;
                    # engines can't shift partitions, so stage + SBUF DMA
                    yt_h = norm.tile([128, 512], bf16, name=f"ytmp{h}{q}", tag="ytmp")
                    nc.vector.tensor_mul(
                        yt_h[64:128, :], yq[64:128, :], rbs[64:128, :]
                    )
                    nc.sync.dma_start(yTn_sb[0:64, m, cols], yt_h[64:128, :])

        # ---- out projection: out[t, e] = y @ Wp_sub^T, head-paired K=128.
        # t-tiles 0..7: pair-3 matmul + staged partials; 8..15: all 4 pairs.
        # psum tiles cycle through all three psum pools (attention is done) for
        # a 6-deep rotation — a 2-slot rotation made the tail latency-bound.
        # psum->sbuf copies alternate scalar/vector so neither engine binds.
        pool_cyc = [(pj, "pj"), (sT_ps, "sT"), (yq_ps, "yq")]
        for mt in range(TM):
            for n in range(2):
                pool, ptag = pool_cyc[(2 * mt + n) % 3]
                ps = pool.tile([128, 512], f32, name=f"pso{mt}{n}", tag=ptag)
                if mt < 4:
                    # fold the bf16 staged partial into psum via an identity
                    # matmul (keeps the tail PE-dense; no cross-engine add)
                    nc.tensor.matmul(
                        ps[:],
                        lhsT=ident_sb[:],
                        rhs=os_part[:, mt, n, :],
                        start=True,
                        stop=False,
                    )
                m0 = 3 if mt < 4 else 0
                for m in range(m0, 4):
                    nc.tensor.matmul(
                        ps[:],
                        lhsT=yTn_sb[:, m, 128 * mt : 128 * mt + 128],
                        rhs=wp_sb[:, m, 512 * n : 512 * n + 512],
                        start=(m == m0) and mt >= 4,
                        stop=(m == 3),
                    )
                o_sb = ost.tile([128, 512], f32, name=f"ost{mt}{n}", tag="ost")
                if (2 * mt + n) % 2:
                    nc.vector.tensor_copy(o_sb[:], ps[:])
                else:
                    nc.scalar.copy(o_sb[:], ps[:])
                nc.sync.dma_start(outr[:, mt, 512 * n : 512 * n + 512], o_sb[:])

    nc.compile()
    return nc


def _get_nc():
    global _NC
    if _NC is None:
        _NC = build_nc()
    return _NC


def kernel(x, Wk, Wq, Wv, Wp, _trace=False):
    from concourse.bass_utils import run_bass_kernel_spmd

    global LAST_RESULTS
    bf16 = ml_dtypes.bfloat16
    x = np.asarray(x, dtype=np.float32)
    Wk = np.asarray(Wk, dtype=np.float32)
    Wq = np.asarray(Wq, dtype=np.float32)
    Wv = np.asarray(Wv, dtype=np.float32)
    Wp = np.asarray(Wp, dtype=np.float32)

    in_maps = []
    for c in range(8):
        b, hh = c // 2, c % 2
        cols = slice(CC * hh, CC * hh + CC)
        in_maps.append(
            {
                "xT": np.ascontiguousarray(x[b].T).astype(bf16),
                "wqT": np.ascontiguousarray(Wq[cols, :].T).astype(bf16),
                "wkT": np.ascontiguousarray(Wk[cols, :].T).astype(bf16),
                "wvT": np.ascontiguousarray(Wv[cols, :].T).astype(bf16),
                "wpT": np.ascontiguousarray(Wp[:, cols].T).astype(bf16),
            }
        )

    nc = _get_nc()
    res = run_bass_kernel_spmd(nc, in_maps, core_ids=list(range(8)), trace=_trace)
    LAST_RESULTS = res

    out = np.empty((B, T, C), dtype=np.float32)
    for b in range(B):
        out[b] = res.results[2 * b]["out"] + res.results[2 * b + 1]["out"]
    return out


if __name__ == "__main__":
    rng = np.random.default_rng(0)
    s = 1.0 / np.sqrt(C)
    inputs = {
        "x": rng.standard_normal((B, T, C), dtype=np.float32),
        "Wk": rng.standard_normal((C, C), dtype=np.float32) * s,
        "Wq": rng.standard_normal((C, C), dtype=np.float32) * s,
        "Wv": rng.standard_normal((C, C), dtype=np.float32) * s,
        "Wp": rng.standard_normal((C, C), dtype=np.float32) * s,
    }
    got = kernel(**inputs)

    def ref(x, Wk, Wq, Wv, Wp):
        def heads(w):
            return (
                np.einsum("btc,ec->bte", x, w)
                .reshape(B, T, N_HEAD, D)
                .transpose(0, 2, 1, 3)
            )

        k, q, v = heads(Wk), heads(Wq), heads(Wv)
        att = np.einsum("bhqd,bhkd->bhqk", q, k) / np.sqrt(D)
        causal = np.tril(np.ones((T, T), dtype=bool))
        att = np.where(causal[None, None], att, -np.inf)
        att = att - att.max(axis=-1, keepdims=True)
        e = np.exp(att)
        p = e / e.sum(axis=-1, keepdims=True)
        y = np.einsum("bhqk,bhkd->bhqd", p, v)
        y = y.transpose(0, 2, 1, 3).reshape(B, T, C)
        return np.einsum("btc,ec->bte", y, Wp)

    want = ref(**{k: v.astype(np.float64) for k, v in inputs.items()}).astype(
        np.float32
    )
    rel = np.linalg.norm(got - want) / np.linalg.norm(want)
    print("rel l2 err:", rel)
    print("max abs err:", np.abs(got - want).max(), "ref absmax:", np.abs(want).max())
